# revision 52
# baseline (speedup 1.0000x reference)
"""Trainium2 Bass kernel for nn_Attention_79671643340898 (CvT-style attention).

Reference computation (per batch element):
  qt/kt/vt = depthwise3x3+BN(x)       [T=784, C=384]
  q/k/v    = qt @ W.T                 [784, 384]
  per head h (6 heads x 64):  S = q_h k_h^T * C**-0.5 ; A = softmax(S)
  o = A v_h ; out = concat(o) @ Wp.T + bp

Strategy: data-parallel over batch (4 images per core x 8 cores).
On-device layout is channel-major ([c, t]); host does all packing/unpacking,
BN folding, and weight transposes. The depthwise conv runs on the PE as 9
accumulated diagonal matmuls per (conv, channel-tile) unit, which keeps the
PE warm and leaves DVE/ACT free for softmax work. Matmul inputs are fp16,
accumulation fp32 in PSUM. Softmax denominator is obtained for free by
appending a ones-column block to V in the A@V matmul; normalization uses a
fast DVE reciprocal.
"""

import sys

for _p in ("/opt/trn_rl_repo", "/root/.axon_site/_ro/trn_rl_repo"):
    if _p not in sys.path:
        sys.path.append(_p)

import numpy as np

B, T, C, NH, HD = 32, 784, 384, 6, 64
H = W = 28
P = 128
CT = 3            # channel tiles of 128
NCORES = 8
IMGS = B // NCORES
SCALE = float(C) ** -0.5
BN_EPS = 1e-5
TT = 7            # t tiles
TS = 112          # t tile size
PADW = 30         # padded image width
XROW = 904        # padded row length (900 used, 904 for slice headroom)
NROW1 = 18        # conv row chunk A: rows 0:18 -> 504 cols (PSUM bank A)
N1 = NROW1 * W    # 504
N2 = T - N1       # 280 (PSUM bank B at offset 512)

_CACHE = {}


def _build_program():
    """Build + compile the Bass program (cached per process)."""
    if "nc" in _CACHE:
        return _CACHE["nc"]
    import concourse.bass as bass
    import concourse.tile as tile
    from concourse import bacc, mybir

    f32 = mybir.dt.float32
    f16 = mybir.dt.float16
    f8 = mybir.dt.float8e4
    DR = mybir.MatmulPerfMode.DoubleRow
    EXP = mybir.ActivationFunctionType.Exp
    MUL = mybir.AluOpType.mult
    ADD = mybir.AluOpType.add

    nc = bacc.Bacc("TRN2", target_bir_lowering=False, debug=False,
                   num_devices=NCORES)

    xpad_d = nc.dram_tensor("xpad", [IMGS, CT, P, XROW], f16,
                            kind="ExternalInput").ap()
    wq_d = nc.dram_tensor("wq", [P, 1152], f16, kind="ExternalInput").ap()
    wk_d = nc.dram_tensor("wk", [P, 1152], f16, kind="ExternalInput").ap()
    wv_d = nc.dram_tensor("wv", [P, 1152], f16, kind="ExternalInput").ap()
    wp_d = nc.dram_tensor("wp", [P, 1152], f16, kind="ExternalInput").ap()
    wd_d = nc.dram_tensor("wd", [P, 81 * P], f16, kind="ExternalInput").ap()
    wc_d = nc.dram_tensor("wc", [P, 81], f32, kind="ExternalInput").ap()
    out_d = nc.dram_tensor("out", [IMGS, CT, P, T], f16,
                           kind="ExternalOutput").ap()

    from contextlib import ExitStack
    with ExitStack() as ctx:
        tc = ctx.enter_context(tile.TileContext(nc))
        pool = lambda **kw: ctx.enter_context(tc.tile_pool(**kw))
        constp = pool(name="const", bufs=1)
        xin = pool(name="xin", bufs=9)
        convp = pool(name="convout", bufs=14)
        qkp = pool(name="qk", bufs=8)
        vpool = pool(name="vp", bufs=9)
        etp = pool(name="et", bufs=18)
        otp = pool(name="ot", bufs=4)
        outp = pool(name="outp", bufs=4)
        stagep = pool(name="stage", bufs=2)
        rtp = pool(name="rt", bufs=4)
        accp = pool(name="acc", bufs=2)
        tmpp = pool(name="tmp", bufs=4)
        # PSUM (8 banks): "conv" gets a dedicated pool (2 banks) so the
        # next image's conv matmuls overlap the current image's
        # ACT-paced attention phase; "st" = S^T tiles (4 banks);
        # "pav" = 1-bank slots for projections / A@V / out (2 banks).
        psconv = pool(name="psconv", bufs=2, space="PSUM")
        psst = pool(name="psst", bufs=2, space="PSUM")
        pspav = pool(name="pspav", bufs=2, space="PSUM")

        # ---- PE warmup ----
        # Dummy matmuls on a memset tile keep the PE busy while the input
        # DMAs land (HAM un-throttles to 2.4 GHz after ~3.4us of activity),
        # so the first conv units run at full clock.
        wsrc = constp.tile([P, 512], f16, tag="warm", name="wsrc")
        nc.gpsimd.memset(wsrc[:], 0.0)

        def dummy_mm(n=1):
            for _ in range(n):
                psd = psconv.tile([P, 512], f32, tag="conv", name="psd")
                nc.tensor.matmul(psd[:, 0:504], wsrc[:, 0:P],
                                 wsrc[:, 0:504], start=True, stop=True)

        dummy_mm(24)

        # ---- load constants ----
        # DMA order matters for startup latency: the first conv unit needs
        # its diag weights + first channel tile of image 0 ASAP.
        wq_s = constp.tile([P, 1152], f16, tag="wq", name="wq_s")
        wk_s = constp.tile([P, 1152], f16, tag="wk", name="wk_s")
        wv_s = constp.tile([P, 1152], f16, tag="wv", name="wv_s")
        wp_s = constp.tile([P, 1152], f16, tag="wp", name="wp_s")
        wd_s = constp.tile([P, 81 * P], f16, tag="wd", name="wd_s")
        wc_s = constp.tile([P, 81], f32, tag="wc", name="wc_s")
        nc.sync.dma_start(wd_s[:, 0:9 * P], wd_d[:, 0:9 * P])
        nc.sync.dma_start(wc_s[:], wc_d[:])
        xp0 = []
        for ct in range(CT):
            t_ = xin.tile([P, XROW], f16, tag="xin", name=f"xp0_{ct}")
            nc.sync.dma_start(t_[:], xpad_d[0, ct])
            xp0.append(t_)
        for u in range(1, 9):
            nc.sync.dma_start(wd_s[:, u * 9 * P:(u + 1) * 9 * P],
                              wd_d[:, u * 9 * P:(u + 1) * 9 * P])
        for d, s in ((wq_d, wq_s), (wk_d, wk_s), (wv_d, wv_s),
                     (wp_d, wp_s)):
            nc.sync.dma_start(s[:], d[:])

        def w_blk(ws, kt, ot):
            return ws[:, (kt * 3 + ot) * P:(kt * 3 + ot + 1) * P]

        def conv_img(img, xp=None):
            """Load one padded image; run the 9 depthwise conv units on the
            PE as 9 accumulated diagonal matmuls each, then cast the PSUM
            result to fp16 in SBUF on the DVE."""
            if xp is None:
                xp = []
                for ct in range(CT):
                    t_ = xin.tile([P, XROW], f16, tag="xin",
                                  name=f"xp{img}_{ct}")
                    nc.sync.dma_start(t_[:], xpad_d[img, ct])
                    xp.append(t_)
            conv_out = [[None] * CT for _ in range(3)]
            for cv in range(3):          # order q -> k -> v
                for ct in range(CT):
                    u = cv * 3 + ct
                    psa = psconv.tile([P, 512], f32, tag="conv", name="psca")
                    psb = psconv.tile([P, 512], f32, tag="conv", name="pscb")
                    for tap in range(9):
                        ky, kx = tap // 3, tap % 3
                        off = ky * PADW + kx
                        src = xp[ct][:, off:off + 840].rearrange(
                            "p (h w) -> p h w", w=PADW)[:, :, 0:W]
                        wd = wd_s[:, (u * 9 + tap) * P:(u * 9 + tap + 1) * P]
                        st, sp = (tap == 0), (tap == 8)
                        nc.tensor.matmul(psa[:, 0:N1], wd,
                                         src[:, 0:NROW1, :],
                                         start=st, stop=sp)
                        nc.tensor.matmul(psb[:, 0:N2], wd,
                                         src[:, NROW1:H, :],
                                         start=st, stop=sp)
                    a = convp.tile([P, T], f16, tag="convout",
                                   name=f"cv{img}_{u}")
                    conv_out[cv][ct] = a
                    nc.vector.tensor_copy(a[:, 0:N1], psa[:, 0:N1])
                    nc.vector.tensor_copy(a[:, N1:T], psb[:, 0:N2])
            return conv_out

        def qk_proj(img, conv_out):
            qk_sb = [[None] * CT, [None] * CT]   # 0: q, 1: k
            for pi, (ws, cvi) in enumerate(((wq_s, 0), (wk_s, 1))):
                for ot in range(CT):
                    sb = qkp.tile([P, T], f16, tag="qk",
                                  name=f"qk{img}_{pi}_{ot}")
                    qk_sb[pi][ot] = sb
                    for c0, cw in ((0, 512), (512, 272)):
                        ps = pspav.tile([P, 512], f32, tag="pav",
                                        name="psqk")
                        for kt in range(CT):
                            nc.tensor.matmul(
                                ps[:, 0:cw], w_blk(ws, kt, ot)[:],
                                conv_out[pi][kt][:, c0:c0 + cw],
                                start=(kt == 0), stop=(kt == CT - 1))
                        nc.vector.tensor_copy(sb[:, c0:c0 + cw], ps[:, 0:cw])
            return qk_sb

        def v_proj(img, conv_out):
            # [t, 6*(64+64)] fp16; cols 0-63 of each head block are ones so
            # A@V replicates the softmax denominator at partitions 0:64
            # (where the DVE reciprocal can read it unshifted); the values
            # land in cols 64-127 -> O at partitions 64:128.
            v_sb = []
            for tt in range(TT):
                sb = vpool.tile([TS, 768], f16, tag="v", name=f"v{img}_{tt}")
                v_sb.append(sb)
                v3 = sb[:].rearrange("p (h d) -> p h d", d=P)
                nc.gpsimd.memset(v3[:, :, 0:64], 1.0)
                ps = pspav.tile([P, 512], f32, tag="pav", name="psv")
                for kt in range(CT):
                    nc.tensor.matmul(
                        ps[0:TS, 0:C],
                        conv_out[2][kt][:, tt * TS:(tt + 1) * TS],
                        wv_s[:, kt * C:(kt + 1) * C],
                        start=(kt == 0), stop=(kt == CT - 1))
                nc.vector.tensor_copy(
                    v3[:, :, 64:P],
                    ps[0:TS, 0:C].rearrange("p (h d) -> p h d", d=64))
            return v_sb

        def attn_pair(img, j, qk_sb, v_sb, oT):
            """Heads 2j, 2j+1: S^T (concurrent via row groups), exp,
            A@V with fused denominator, normalize."""
            et = [[None] * TT, [None] * TT]
            for tt in range(TT):
                pse = psst.tile([TS, T], f32, tag="st", name="pse")
                pso = psst.tile([TS, T], f32, tag="st", name="pso")
                for c0, cw in ((0, 512), (512, 272)):
                    for hh, ps in ((0, pse), (1, pso)):
                        sl = slice(64 * hh, 64 * hh + 64)
                        nc.tensor.matmul(
                            ps[:, c0:c0 + cw],
                            qk_sb[1][j][sl, tt * TS:(tt + 1) * TS],
                            qk_sb[0][j][sl, c0:c0 + cw],
                            start=True, stop=True)
                for hh, ps in ((0, pse), (1, pso)):
                    e = etp.tile([TS, T], f16, tag="et",
                                 name=f"et{img}_{j}_{hh}_{tt}")
                    et[hh][tt] = e
                    nc.scalar.activation(e[:], ps[:], EXP, scale=SCALE)
                if img == IMGS - 1:
                    # no next-image conv to fill the exp-paced tail; keep
                    # the PE warm with throwaway matmuls instead.
                    dummy_mm(3)
            for hh in range(2):
                h = 2 * j + hh
                pa = pspav.tile([P, 512], f32, tag="pav", name="psavA")
                pb = pspav.tile([P, 512], f32, tag="pav", name="psavB")
                for tt in range(TT):
                    lhs = v_sb[tt][:, P * h:P * h + P]
                    st, sp = (tt == 0), (tt == TT - 1)
                    nc.tensor.matmul(pa[:, 0:512], lhs,
                                     et[hh][tt][:, 0:512], start=st, stop=sp)
                    nc.tensor.matmul(pb[:, 0:272], lhs,
                                     et[hh][tt][:, 512:784], start=st,
                                     stop=sp)
                # denominator at partitions 0:64, O at 64:128; per-chunk
                # recip->DMA->normalize chains so each PSUM slot frees as
                # early as possible.
                rlo = rtp.tile([64, T], f32, tag="rlo", name="rlo")
                rhi = rtp.tile([P, T], f32, tag="rhi", name="rhi")
                dest = (oT[j] if hh == 1 else
                        stagep.tile([P, T], f16, tag="stage", name="stg"))
                for ps, c0, cw in ((pa, 0, 512), (pb, 512, 272)):
                    nc.vector.reciprocal_approx_fast(rlo[:, c0:c0 + cw],
                                                     ps[0:64, 0:cw])
                    nc.sync.dma_start(rhi[64:P, c0:c0 + cw],
                                      rlo[:, c0:c0 + cw])
                    nc.vector.tensor_tensor(dest[64:P, c0:c0 + cw],
                                            ps[64:P, 0:cw],
                                            rhi[64:P, c0:c0 + cw], op=MUL)
                if hh == 0:
                    nc.sync.dma_start(oT[j][0:64, :], dest[64:P, :])
                if img == IMGS - 1:
                    dummy_mm(3)

        def out_proj(img, oT):
            for ot in range(CT):
                osb = outp.tile([P, T], f16, tag="out",
                                name=f"osb{img}_{ot}")
                for c0, cw in ((0, 512), (512, 272)):
                    ps = pspav.tile([P, 512], f32, tag="pav",
                                    name="psout")
                    for kt in range(CT):
                        nc.tensor.matmul(
                            ps[:, 0:cw], w_blk(wp_s, kt, ot)[:],
                            oT[kt][:, c0:c0 + cw],
                            start=(kt == 0), stop=(kt == CT - 1))
                    nc.vector.tensor_copy(osb[:, c0:c0 + cw], ps[:, 0:cw])
                nc.sync.dma_start(out_d[img, ot], osb[:])
                if img == IMGS - 1:
                    dummy_mm(2)

        for img in range(IMGS):
            if img > 0:
                # bias the scheduler to slot conv work ahead of
                # exp-gated attention matmuls (which head-block the
                # in-order PE stream when placed too early)
                with tc.high_priority():
                    conv_out = conv_img(img)
            else:
                conv_out = conv_img(img, xp0)
            qk_sb = qk_proj(img, conv_out)
            v_sb = v_proj(img, conv_out)
            oT = [otp.tile([P, T], f16, tag="ot", name=f"oT{img}_{i}")
                  for i in range(CT)]
            for j in range(CT):
                attn_pair(img, j, qk_sb, v_sb, oT)
            out_proj(img, oT)

    nc.compile()
    _CACHE["nc"] = nc
    return nc


def _prep_inputs(inputs):
    """Host-side packing: returns (in_maps list per core)."""
    x = np.asarray(inputs["x"], np.float32)

    def fold(nm):
        inv = (np.asarray(inputs[f"gamma_{nm}"], np.float32)
               / np.sqrt(np.asarray(inputs[f"var_{nm}"], np.float32) + BN_EPS))
        wc = (np.asarray(inputs[f"conv_w_{nm}"], np.float32)
              .reshape(C, 9) * inv[:, None])
        bias_eff = (np.asarray(inputs[f"beta_{nm}"], np.float32)
                    - np.asarray(inputs[f"mean_{nm}"], np.float32) * inv)
        return wc, bias_eff

    wc_q, be_q = fold("q")
    wc_k, be_k = fold("k")
    wc_v, be_v = fold("v")
    w_q = np.asarray(inputs["w_q"], np.float32)
    w_k = np.asarray(inputs["w_k"], np.float32)
    w_v = np.asarray(inputs["w_v"], np.float32)
    w_p = np.asarray(inputs["w_proj"], np.float32)
    b_p = np.asarray(inputs["b_proj"], np.float32)
    qb, kb, vb = w_q @ be_q, w_k @ be_k, w_v @ be_v
    assert (np.abs(qb).max() == 0 and np.abs(kb).max() == 0
            and np.abs(vb).max() == 0 and np.abs(b_p).max() == 0), \
        "nonzero folded biases not supported by compiled program"

    # weight packing
    def pack_lhsT(w):
        # [128, (kt,ot,c_out_loc)] : value = w[ot*128+j, kt*128+i]
        out = np.empty((P, 1152), np.float32)
        for kt in range(CT):
            for ot in range(CT):
                blk = w[ot * P:(ot + 1) * P, kt * P:(kt + 1) * P]  # [j, i]
                out[:, (kt * 3 + ot) * P:(kt * 3 + ot + 1) * P] = blk.T
        return out.astype(np.float16)

    wq_h = pack_lhsT(w_q)
    wk_h = pack_lhsT(w_k)
    wp_h = pack_lhsT(w_p)
    wv_h = np.empty((P, 1152), np.float32)
    for kt in range(CT):
        wv_h[:, kt * C:(kt + 1) * C] = w_v[:, kt * P:(kt + 1) * P].T
    wv_h = wv_h.astype(np.float16)

    # conv weights as 81 diagonal [128, 128] blocks (unit-major, tap-minor)
    # plus the raw per-channel weights for the vector-engine conv unit
    wd_h = np.zeros((P, 81 * P), np.float32)
    wc_h = np.empty((P, 81), np.float32)
    ar = np.arange(P)
    for cv, wc in enumerate((wc_q, wc_k, wc_v)):
        for ct in range(CT):
            for tap in range(9):
                blk = (cv * 3 + ct) * 9 + tap
                wd_h[ar, blk * P + ar] = wc[ct * P:(ct + 1) * P, tap]
            wc_h[:, (cv * 3 + ct) * 9:(cv * 3 + ct + 1) * 9] = \
                wc[ct * P:(ct + 1) * P]
    wd_h = wd_h.astype(np.float16)

    # padded images, channel-major, fp16
    xt = x.reshape(B, H, W, C).transpose(0, 3, 1, 2)  # [B, C, H, W]
    xpad = np.zeros((B, C, PADW, PADW), np.float32)
    xpad[:, :, 1:29, 1:29] = xt
    xpad = xpad.reshape(B, C, 900).astype(np.float16)
    xrow = np.zeros((B, CT, P, XROW), np.float16)
    for ct in range(CT):
        xrow[:, ct, :, 0:900] = xpad[:, ct * P:(ct + 1) * P]
    in_maps = []
    for core in range(NCORES):
        in_maps.append({
            "xpad": xrow[core * IMGS:(core + 1) * IMGS],
            "wq": wq_h, "wk": wk_h, "wv": wv_h, "wp": wp_h, "wd": wd_h,
            "wc": wc_h,
        })
    return in_maps


def _run(inputs, trace=False, tmpdir=None):
    from concourse import bass_utils
    nc = _build_program()
    in_maps = _prep_inputs(inputs)
    res = bass_utils.run_bass_kernel_spmd(
        nc, in_maps, core_ids=list(range(NCORES)), trace=trace,
        tmpdir=tmpdir)
    # gather: out [IMGS, CT, 128, T] per core -> [B, T, C]
    out = np.empty((B, T, C), np.float32)
    for core in range(NCORES):
        o = np.asarray(res.results[core]["out"], np.float32)  # [IMGS,CT,P,T]
        for i in range(IMGS):
            out[core * IMGS + i] = o[i].reshape(C, T).T
    return out, res


def kernel(**inputs):
    out, _ = _run(inputs)
    return out


def kernel_with_stats(trace=True, tmpdir=None, **inputs):
    out, res = _run(inputs, trace=trace, tmpdir=tmpdir)
    return out, res
